# revision 12
# baseline (speedup 1.0000x reference)
# Causal self-attention kernel for 8 Trainium2 NeuronCores (Bass/Tile).
#
# Sharding: core c -> batch b = c//4, head group g = c%4 (heads 4g..4g+3).
# Each core computes the qkv projection for its batch restricted to its heads
# (column-sharded Wqkv), rope, causal flash attention for its 4 heads, and a
# row-sharded output projection producing a partial [S, D] bf16 output.  The
# host sums the 4 partials per batch (f32) and adds bout.
#
# Schedule: the scalar (ACT) engine's exp stream is the phase-B bottleneck
# (~94us), so attention starts as early as possible -- right after the q01/k01
# projection groups -- and the remaining projection work (v chunks, q23, k23)
# is emitted as PE "filler" between the scores and PV matmuls of heads 0/1.
# This feeds the exp stream ~35us earlier and hides the ACT-vs-PE rate gap.
# y tiles 0..7 fill head 3's second half the same way; 8..15 drain at the end.
#
# Device-side notes:
#  * All matmul inputs are bf16; accumulation f32 in PSUM.
#  * x pre-transposed on host to xT [D, S]; q/k produced transposed with
#    per-head dims permuted to [evens(32), odds(32)]; rope swap32 is 4
#    SBUF->SBUF DMAs split across the sync/gpsimd queues, rot = raw*P + swp*Q.
#  * Scores transposed sT[k, q]; k-side stationary zero-padded to K=128
#    (K=64 matmuls never warm the PE clock gate and run at half clock).
#  * Causal diag masking: multiply p by a 0/1 lower-tri tile on the DVE after
#    the exp (bf16 SBUF, cheap) -- no PE mask matmuls.
#  * Softmax without max-subtraction: p = exp(s/8) from PSUM on ACT, bf16.
#  * v_ext [k, 65] carries a ones-column so PV accumulates the softmax
#    denominator as row 64 of oT [65, 1024].  oT is evacuated to SBUF
#    immediately (num+den copies) so the single-buffered oT bank pair frees
#    fast; reciprocal_approx_fast (SBUF only! it mis-reads partition-offset
#    PSUM) + gpsimd partition_broadcast + one multiply normalize into aT.
#  * PSUM: ps_s 2x[128,1024] (4 banks) + ps_o 1x[65,1024] (2) + ps_fill
#    2x[128,512] (2) = 8; ps_fill closes after the fillers drain and ps_y
#    (2 banks) takes its place for the output projection.
#  * Output projection y[q, n] = sum_d aT[d, q] Wout[d, n], q-tiled, aT
#    stationary reused across both 512-col halves, bf16 output.

import numpy as np
import ml_dtypes

import concourse.bass as bass
import concourse.tile as tile
import concourse.mybir as mybir
from concourse import bacc
from concourse.bass import ts, ds
from concourse.bass_utils import run_bass_kernel_spmd

BF16 = mybir.dt.bfloat16
F32 = mybir.dt.float32
AF = mybir.ActivationFunctionType
ALU = mybir.AluOpType

B, S, D = 2, 2048, 1024
H, HD = 16, 64
NCORES = 8
HPC = 4            # heads per core
NT = S // 128      # 16 seq tiles
SCALE = HD ** -0.5

# Module-level knobs / results (used by test.py).
TRACE = False
LAST_RESULTS = None


def _body(ctx, tc, ins, outs):
    nc = tc.nc
    xT, wqk, wv, wout, bqk, bvb, ropeP, ropeQ = ins
    (y,) = outs

    # ---- SBUF pools ----
    p_x = ctx.enter_context(tc.tile_pool(name="x", bufs=1))
    p_w = ctx.enter_context(tc.tile_pool(name="w", bufs=1))
    p_cst = ctx.enter_context(tc.tile_pool(name="cst", bufs=1))
    p_qk = ctx.enter_context(tc.tile_pool(name="qk", bufs=1))
    p_vx = ctx.enter_context(tc.tile_pool(name="vx", bufs=1))
    p_aT = ctx.enter_context(tc.tile_pool(name="aT", bufs=1))
    p_tmp = ctx.enter_context(tc.tile_pool(name="tmp", bufs=3))
    p_p = ctx.enter_context(tc.tile_pool(name="p", bufs=6))
    p_r = ctx.enter_context(tc.tile_pool(name="r", bufs=2))
    p_y = ctx.enter_context(tc.tile_pool(name="y", bufs=3))

    # ---- load inputs (x and weight DMAs interleaved) ----
    # x loads split into column halves: the first halves are all q01/k01
    # ns 0,1 need, so they land ~2x sooner; weight issue order puts wqk +
    # rope tables + biases (everything the pre-attention region touches)
    # ahead of wv/wout on the sync queue.
    x_sb, wqk_sb, wv_sb = [], [], []
    for kc in range(8):
        t = p_x.tile([128, S], BF16, tag=f"x{kc}", name=f"x{kc}")
        nc.scalar.dma_start(t[:, 0:1024], xT[ts(kc, 128), 0:1024])
        x_sb.append(t)
        t = p_w.tile([128, 512], BF16, tag=f"wqk{kc}", name=f"wqk{kc}")
        nc.sync.dma_start(t[:, :], wqk[ts(kc, 128), :])
        wqk_sb.append(t)
    tabP = p_cst.tile([128, S], BF16, tag="tabP")
    nc.sync.dma_start(tabP[:, :], ropeP[:, :])
    tabQ = p_cst.tile([128, S], BF16, tag="tabQ")
    nc.sync.dma_start(tabQ[:, :], ropeQ[:, :])
    bqk_sb = []
    for mc in range(4):
        t = p_cst.tile([128, 1], F32, tag=f"bqk{mc}", name=f"bqk{mc}")
        nc.sync.dma_start(t[:, :], bqk[ts(mc, 128)].rearrange("(p o) -> p o", o=1))
        bqk_sb.append(t)
    for kc in range(8):
        nc.scalar.dma_start(x_sb[kc][:, 1024:2048], xT[ts(kc, 128), 1024:2048])
        t = p_w.tile([128, 256], BF16, tag=f"wv{kc}", name=f"wv{kc}")
        nc.sync.dma_start(t[:, :], wv[ts(kc, 128), :])
        wv_sb.append(t)
    wout_sb = []
    for kc in range(2):
        t = p_w.tile([128, D], BF16, tag=f"wout{kc}", name=f"wout{kc}")
        nc.sync.dma_start(t[:, :], wout[ts(kc, 128), :])
        wout_sb.append(t)
    bvb_sb = p_cst.tile([128, 256], F32, tag="bvb")
    nc.sync.dma_start(bvb_sb[:, :], bvb[:, :])

    # 0/1 lower-triangle (keep where q >= k) for post-exp diag masking.
    ones_t = p_cst.tile([128, 128], BF16, tag="ones")
    nc.vector.memset(ones_t[:, :], 1.0)
    causal01 = p_cst.tile([128, 128], BF16, tag="causal01")
    nc.gpsimd.affine_select(causal01[:, :], ones_t[:, :], pattern=[[1, 128]],
                            compare_op=ALU.is_ge, fill=0.0, base=0,
                            channel_multiplier=-1)  # keep where q - k >= 0

    qk_sb = []   # [q01, q23, k01, k23], bf16 [128, S] each (post-rope)
    for mc in range(4):
        qk_sb.append(p_qk.tile([128, S], BF16, tag=f"qkT{mc}", name=f"qkT{mc}"))
    kpad_sb = []
    for h in range(HPC):
        t = p_qk.tile([128, S], BF16, tag=f"kpad{h}", name=f"kpad{h}")
        nc.vector.memset(t[64 * (1 - h % 2):64 * (1 - h % 2) + 64, :], 0.0)
        kpad_sb.append(t)
    vx_sb = [None] * NT  # [128, 4*65] bf16: per head 64 v-cols + ones col
    aT_sb = [p_aT.tile([128, S], BF16, tag=f"aT{i}", name=f"aT{i}")
             for i in range(2)]

    def rope(mc, ns, qk_ps):
        raw = p_tmp.tile([128, 512], BF16, tag="raw")
        nc.vector.tensor_scalar_add(raw[:, :], qk_ps[:, :], bqk_sb[mc][:, :])
        swp = p_tmp.tile([128, 512], BF16, tag="swp")
        for i, (blk, sb) in enumerate(((0, 32), (32, 0), (64, 96), (96, 64))):
            eng = nc.sync if i % 2 == 0 else nc.gpsimd
            eng.dma_start(swp[ds(blk, 32), :], raw[ds(sb, 32), :])
        t2 = p_tmp.tile([128, 512], BF16, tag="t2")
        nc.vector.tensor_mul(t2[:, :], raw[:, :], tabP[:, ts(ns, 512)])
        tq = p_tmp.tile([128, 512], BF16, tag="tq")
        nc.vector.tensor_mul(tq[:, :], swp[:, :], tabQ[:, ts(ns, 512)])
        if mc < 2:
            nc.vector.tensor_add(qk_sb[mc][:, ts(ns, 512)], t2[:, :], tq[:, :])
        else:
            hp = mc - 2
            for hr in range(2):
                nc.vector.tensor_add(
                    kpad_sb[2 * hp + hr][ds(64 * hr, 64), ts(ns, 512)],
                    t2[ds(64 * hr, 64), :], tq[ds(64 * hr, 64), :])

    pend = [None]   # (p_t, j, q0, w, oT, qlo, h)

    def emit_pv(pv):
        p_t, j, q0, w, oT, qlo, h = pv
        c0 = (q0 - qlo) * 128
        pos = c0
        while pos < c0 + w:
            nxt = min((pos // 512 + 1) * 512, c0 + w)
            gbank = (qlo * 128 + pos) // 512
            nc.tensor.matmul(
                oT[:, ds(pos, nxt - pos)],
                vx_sb[j][:, ds(65 * h, 65)],
                p_t[:, ds(pos - c0, nxt - pos)],
                start=(j == 0), stop=(j == 4 * gbank + 3),
                skip_group_check=True)
            pos = nxt

    def finish_half(h, hp, hr, half, oT):
        # Evacuate oT to SBUF right away (num + den copies) so the single-
        # buffered oT bank pair frees fast; normalize off the critical path.
        num = p_r.tile([64, 1024], BF16, tag="num", name=f"num{h}_{half}")
        nc.vector.tensor_copy(num[:, :], oT[0:64, :])
        den = p_r.tile([1, 1024], F32, tag="den", name=f"den{h}_{half}")
        nc.vector.tensor_copy(den[:, :], oT[64:65, :])
        rden = p_r.tile([1, 1024], F32, tag="rden", name=f"rden{h}_{half}")
        nc.vector.reciprocal_approx_fast(rden[:, :], den[:, :])
        rb = p_r.tile([64, 1024], F32, tag="rb", name=f"rb{h}_{half}")
        nc.gpsimd.partition_broadcast(rb[:, :], rden[:, :])
        nc.vector.tensor_mul(
            aT_sb[hp][ds(64 * hr, 64), ds(1024 * half, 1024)],
            num[:, :], rb[:, :])

    def attn_half(h, half, ps_s, ps_o, finishes, pump):
        hp, hr = h // 2, h % 2
        qT = qk_sb[hp]
        kT = kpad_sb[h]
        qlo, qhi = 8 * half, 8 * half + 8   # q-tile range
        oT = ps_o.tile([65, 1024], F32, tag="oT", name=f"oT{h}_{half}")
        for j in range(qhi):
            q0 = max(j, qlo)
            w = (qhi - q0) * 128
            s_ps = ps_s.tile([128, 1024], F32, tag="s")
            for n0 in range(0, w, 512):
                nn = min(512, w - n0)
                nc.tensor.matmul(
                    s_ps[:, ds(n0, nn)],
                    kT[:, ts(j, 128)],
                    qT[:, ds(q0 * 128 + n0, nn)],
                    start=True, stop=True,
                    skip_group_check=True)
            p_t = p_p.tile([128, 1024], BF16, tag="p")
            nc.scalar.activation(
                p_t[:, 0:w], s_ps[:, 0:w], AF.Exp, scale=SCALE)
            if q0 == j:
                nc.vector.tensor_mul(p_t[:, 0:128], p_t[:, 0:128],
                                     causal01[:, :])
            pump(j)   # PE filler lands between scores(j)/exp(j) and PV(j-1)
            if pend[0] is not None:
                emit_pv(pend[0])
                for fin in finishes:
                    fin()
                finishes.clear()
            pend[0] = (p_t, j, q0, w, oT, qlo, h)
        finishes.append(
            lambda h=h, hp=hp, hr=hr, half=half, oT=oT:
                finish_half(h, hp, hr, half, oT))

    def flush(finishes):
        if pend[0] is not None:
            emit_pv(pend[0])
            pend[0] = None
        for fin in finishes:
            fin()
        finishes.clear()

    def y_tile(qt, ps_y):
        y_pss = [ps_y.tile([128, 512], F32, tag="y", name=f"y{qt}_{i}")
                 for i in range(2)]
        for kc in range(2):
            for nh in range(2):
                nc.tensor.matmul(
                    y_pss[nh][:, :],
                    aT_sb[kc][:, ts(qt, 128)],
                    wout_sb[kc][:, ts(nh, 512)],
                    start=(kc == 0), stop=(kc == 1),
                    skip_group_check=True)
        y_sb = p_y.tile([128, 1024], BF16, tag="ysb")
        nc.vector.tensor_copy(y_sb[:, 0:512], y_pss[0][:, :])
        nc.scalar.copy(y_sb[:, 512:1024], y_pss[1][:, :])
        nc.sync.dma_start(y[ts(qt, 128), :], y_sb[:, :])

    finishes = []
    with tc.tile_pool(name="ps_s", bufs=2, space="PSUM") as ps_s, \
         tc.tile_pool(name="ps_o", bufs=1, space="PSUM") as ps_o:
        fillers = []
        v_done = [0]
        pace = [0]
        with tc.tile_pool(name="ps_fill", bufs=2, space="PSUM") as ps_fill:
            with nc.named_scope("warmup"):
                # dense K=128 matmuls (discarded) flip the PE clock gate to
                # 2.4 GHz while the x DMAs are still in flight
                wu = ps_fill.tile([128, 512], F32, tag="fill", name="warmup")
                for r in range(18):
                    nc.tensor.matmul(wu[:, :], wqk_sb[0][:, 0:128],
                                     wqk_sb[0][:, :], start=(r == 0),
                                     stop=(r == 17), skip_group_check=True)

            def qk_ns(mc, ns):
                qk_ps = ps_fill.tile([128, 512], F32, tag="fill",
                                     name=f"qk{mc}_{ns}")
                for kc in range(8):
                    nc.tensor.matmul(
                        qk_ps[:, :],
                        wqk_sb[kc][:, ts(mc, 128)],
                        x_sb[kc][:, ts(ns, 512)],
                        start=(kc == 0), stop=(kc == 7))
                rope(mc, ns, qk_ps)

            def v_chunk(st):
                v_ps = ps_fill.tile([128, 256], F32, tag="fill",
                                    name=f"v{st}")
                for kc in range(8):
                    nc.tensor.matmul(
                        v_ps[:, :],
                        x_sb[kc][:, ts(st, 128)],
                        wv_sb[kc][:, :],
                        start=(kc == 0), stop=(kc == 7))
                vx_t = p_vx.tile([128, HPC * 65], BF16, tag=f"vx{st}",
                                 name=f"vx{st}")
                vv = vx_t.rearrange("p (h c) -> p h c", c=65)
                nc.vector.memset(vv[:, :, 64:65], 1.0)
                nc.vector.tensor_add(
                    vv[:, :, 0:64],
                    v_ps.rearrange("p (h c) -> p h c", c=64)[:, :, :],
                    bvb_sb.rearrange("p (h c) -> p h c", c=64)[:, :, :])
                vx_sb[st] = vx_t
                v_done[0] += 1

            with nc.named_scope("qk01"):
                # h0.half0 only touches q/k columns 0:1024 (ns 0,1) -- emit
                # just those before attention starts; ns 2,3 lead the fillers
                # (the mandatory-v pump drains the prefix early, well before
                # h0.half1 needs them).
                for ns in range(2):
                    qk_ns(0, ns)   # q heads 0/1
                for ns in range(2):
                    qk_ns(2, ns)   # k heads 0/1 -> kpad0/1
            for mc in (0, 2):
                for ns in range(2, 4):
                    fillers.append(lambda mc=mc, ns=ns: qk_ns(mc, ns))
            for st in range(NT):
                fillers.append(lambda st=st: v_chunk(st))
            for mc in (1, 3):      # q23, then k23 (heads 2/3)
                for ns in range(4):
                    fillers.append(lambda mc=mc, ns=ns: qk_ns(mc, ns))

            def pump(j):
                # mandatory: keep v chunks 2 ahead of the PV consumer
                while fillers and v_done[0] < min(j + 2, NT):
                    fillers.pop(0)()
                pace[0] += 1
                if fillers and pace[0] % 2 == 0:
                    fillers.pop(0)()

            for h in (0, 1):
                with nc.named_scope(f"attn_h{h}"):
                    for half in range(2):
                        attn_half(h, half, ps_s, ps_o, finishes, pump)
            while fillers:   # h2/h3 need q23/k23 done
                fillers.pop(0)()
        with tc.tile_pool(name="ps_y", bufs=2, space="PSUM") as ps_y:
            nop = lambda j: None
            with nc.named_scope("attn_h2"):
                for half in range(2):
                    attn_half(2, half, ps_s, ps_o, finishes, nop)
            with nc.named_scope("attn_h3a"):
                attn_half(3, 0, ps_s, ps_o, finishes, nop)
            yq = [0]

            def ypump(j):
                if j % 2 == 1 and yq[0] < 8:
                    y_tile(yq[0], ps_y)
                    yq[0] += 1

            with nc.named_scope("attn_h3b"):
                attn_half(3, 1, ps_s, ps_o, finishes, ypump)
            flush(finishes)
            with nc.named_scope("y_proj_b"):
                for qt in range(yq[0], NT):
                    y_tile(qt, ps_y)


def build():
    nc = bacc.Bacc("TRN2", target_bir_lowering=False, debug=False,
                   num_devices=NCORES)
    xT = nc.dram_tensor("xT", [D, S], BF16, kind="ExternalInput").ap()
    wqk = nc.dram_tensor("wqk", [D, 512], BF16, kind="ExternalInput").ap()
    wv = nc.dram_tensor("wv", [D, 256], BF16, kind="ExternalInput").ap()
    wout = nc.dram_tensor("wout", [256, D], BF16, kind="ExternalInput").ap()
    bqk = nc.dram_tensor("bqk", [512], F32, kind="ExternalInput").ap()
    bvb = nc.dram_tensor("bvb", [128, 256], F32, kind="ExternalInput").ap()
    ropeP = nc.dram_tensor("ropeP", [128, S], BF16, kind="ExternalInput").ap()
    ropeQ = nc.dram_tensor("ropeQ", [128, S], BF16, kind="ExternalInput").ap()
    y = nc.dram_tensor("y", [S, D], BF16, kind="ExternalOutput").ap()

    from contextlib import ExitStack
    with tile.TileContext(nc) as tc:
        with ExitStack() as ctx:
            _body(ctx, tc, (xT, wqk, wv, wout, bqk, bvb, ropeP, ropeQ), (y,))
    nc.compile()
    return nc


_EVEN_ODD = np.concatenate([np.arange(0, HD, 2), np.arange(1, HD, 2)])


def make_core_inputs(x, rope_cos, rope_sin, Wqkv, bqkv, Wout, bout, core):
    """Build the per-core device input map (numpy, host-side sharding)."""
    b, g = core // HPC, core % HPC
    heads = [HPC * g + i for i in range(HPC)]
    bf = ml_dtypes.bfloat16

    xT = np.ascontiguousarray(x[b].T).astype(bf)

    # wqk columns: [q01, q23, k01, k23]; within each head [evens, odds]
    qcols, kcols = [], []
    for h in heads:
        qcols.append(Wqkv[:, 0 * D + 64 * h + _EVEN_ODD])
        kcols.append(Wqkv[:, 1 * D + 64 * h + _EVEN_ODD])
    wqk_np = np.concatenate(
        [qcols[0], qcols[1], qcols[2], qcols[3],
         kcols[0], kcols[1], kcols[2], kcols[3]], axis=1)
    bq = [bqkv[0 * D + 64 * h + _EVEN_ODD] for h in heads]
    bk = [bqkv[1 * D + 64 * h + _EVEN_ODD] for h in heads]
    bqk_np = np.concatenate([bq[0], bq[1], bq[2], bq[3],
                             bk[0], bk[1], bk[2], bk[3]])

    wv_np = np.concatenate(
        [Wqkv[:, 2 * D + 64 * h:2 * D + 64 * h + 64] for h in heads], axis=1)
    bv = np.concatenate(
        [bqkv[2 * D + 64 * h:2 * D + 64 * h + 64] for h in heads])
    bvb_np = np.tile(bv[None, :], (128, 1)).astype(np.float32)

    wout_np = np.concatenate(
        [Wout[64 * h:64 * h + 64, :] for h in heads], axis=0)

    cosT = np.ascontiguousarray(rope_cos.T).astype(np.float32)  # [32, S]
    sinT = np.ascontiguousarray(rope_sin.T).astype(np.float32)
    ropeP_np = np.tile(np.concatenate([cosT, cosT], axis=0), (2, 1))
    ropeQ_np = np.tile(np.concatenate([-sinT, sinT], axis=0), (2, 1))

    return {
        "xT": xT,
        "wqk": np.ascontiguousarray(wqk_np).astype(bf),
        "wv": np.ascontiguousarray(wv_np).astype(bf),
        "wout": np.ascontiguousarray(wout_np).astype(bf),
        "bqk": bqk_np.astype(np.float32),
        "bvb": bvb_np,
        "ropeP": np.ascontiguousarray(ropeP_np).astype(bf),
        "ropeQ": np.ascontiguousarray(ropeQ_np).astype(bf),
    }


_NC_CACHE = None


def kernel(x, rope_cos, rope_sin, Wqkv, bqkv, Wout, bout):
    global _NC_CACHE, LAST_RESULTS
    x = np.asarray(x, dtype=np.float32)
    rope_cos = np.asarray(rope_cos, dtype=np.float32)
    rope_sin = np.asarray(rope_sin, dtype=np.float32)
    Wqkv = np.asarray(Wqkv, dtype=np.float32)
    bqkv = np.asarray(bqkv, dtype=np.float32)
    Wout = np.asarray(Wout, dtype=np.float32)
    bout = np.asarray(bout, dtype=np.float32)

    if _NC_CACHE is None:
        _NC_CACHE = build()
    nc = _NC_CACHE

    in_maps = [
        make_core_inputs(x, rope_cos, rope_sin, Wqkv, bqkv, Wout, bout, c)
        for c in range(NCORES)
    ]
    res = run_bass_kernel_spmd(nc, in_maps, core_ids=list(range(NCORES)),
                               trace=TRACE)
    LAST_RESULTS = res

    out = np.zeros((B, S, D), dtype=np.float32)
    for c in range(NCORES):
        out[c // HPC] += res.results[c]["y"].astype(np.float32)
    out += bout[None, None, :]
    return out
